# revision 8
# baseline (speedup 1.0000x reference)
"""Self-contained Trainium2 kernel for nn_DynamicConv2D (moe_routing).

Contract: kernel(**inputs) takes FULL unsharded inputs (numpy), returns the
FULL output [32, 64, 64, 128] float32. Internally shards batch across 8
NeuronCores (4 samples each), runs a Bass/Tile kernel via
run_bass_kernel_spmd, and gathers.

The routing network (global pool -> reduce -> relu -> softmax) and the
expert-kernel mixing are tiny (~19M MACs for the whole batch) and run on the
host in fp32; BN scale/bias and the routed bias are folded in there too.
The device kernel is then a pure per-sample 3x3 conv:

  conv  = 9-tap shifted fp16 matmuls accumulated in PSUM, per 512-pos chunk
  out   = Relu(conv + beta)   (ACT epilogue with per-partition bias, fp16 out)

Layout: x is host-transposed to channel-major [C, H, W], zero-padded to
[C, 66, 66], and cast to fp16 so all 9 conv taps are plain access-pattern
offsets; output is produced channel-major [F, H*W] fp16 and host-transposed
back to NHWC f32. Mixed per-sample weights [C, 9*F] fp16 are DMA'd per
sample (same bytes as the expert bank).
"""

import os
import sys

if "/opt/trn_rl_repo" not in sys.path:
    sys.path.insert(0, "/opt/trn_rl_repo")
if not os.environ.get("JAX_PLATFORMS"):
    os.environ["JAX_PLATFORMS"] = "axon"

import numpy as np

import concourse.bacc as bacc
import concourse.tile as tile
from concourse import mybir
from concourse.bass_utils import run_bass_kernel_spmd


def _ensure_ntff_hook():
    """run_bass_kernel_spmd(trace=True) under axon needs antenv.axon_hooks,
    which this image's antenv package lacks. Register an equivalent module
    (ctypes into libaxon_pjrt.so) so profiled runs work."""
    try:
        from antenv import axon_hooks  # noqa: F401
        return
    except ImportError:
        pass
    import contextlib
    import ctypes
    import types

    so_path = os.environ.get("AXON_PJRT_SO", "/opt/axon/libaxon_pjrt.so")
    mod = types.ModuleType("antenv.axon_hooks")
    state = {"hook": None}

    def _make_hook():
        if not os.path.exists(so_path):
            return None
        lib = ctypes.CDLL(so_path)
        if not hasattr(lib, "axon_start_nrt_profile"):
            return None
        lib.axon_start_nrt_profile.argtypes = [
            ctypes.POINTER(ctypes.c_int64), ctypes.c_size_t]
        lib.axon_start_nrt_profile.restype = ctypes.c_int64
        lib.axon_stop_nrt_profile.argtypes = [ctypes.c_char_p]
        lib.axon_stop_nrt_profile.restype = ctypes.c_int64

        @contextlib.contextmanager
        def _hook(output_dir, device_ids):
            import jax
            jax.devices()
            if device_ids:
                ids = (ctypes.c_int64 * len(device_ids))(*device_ids)
                rc = lib.axon_start_nrt_profile(ids, len(device_ids))
            else:
                rc = lib.axon_start_nrt_profile(None, 0)
            if rc != 0:
                raise RuntimeError(f"axon_start_nrt_profile rc={rc}")
            try:
                yield
            finally:
                n = lib.axon_stop_nrt_profile(str(output_dir).encode())
                if n < 0:
                    raise RuntimeError(f"axon_stop_nrt_profile rc={n}")

        return _hook

    def get_axon_ntff_profile_hook():
        if state["hook"] is None:
            state["hook"] = _make_hook()
        return state["hook"]

    def set_axon_ntff_profile_hook(hook):
        state["hook"] = hook

    mod.get_axon_ntff_profile_hook = get_axon_ntff_profile_hook
    mod.set_axon_ntff_profile_hook = set_axon_ntff_profile_hook
    sys.modules["antenv.axon_hooks"] = mod
    try:
        import antenv
        antenv.axon_hooks = mod
    except ImportError:
        pass


F32 = mybir.dt.float32
F16 = mybir.dt.float16
AF = mybir.ActivationFunctionType

B, H, W, C = 32, 64, 64, 128
NCORES = 8
BPC = B // NCORES  # samples per core
HP, WP = H + 2, W + 2  # zero-padded
NPAD = HP * WP  # 4356
NPOS = H * W  # 4096
K = 4  # experts
NF = 128  # output filters
TAPS = 9
WMC = TAPS * NF  # wm columns per sample (1152)
ROWS_PER_CHUNK = 8  # 8 image rows * 64 cols = 512 positions per PSUM chunk
NCHUNK = H // ROWS_PER_CHUNK
X_P0A = 10 * WP  # sample-0 input piece A: rows 0..9 (covers conv chunk 0)
X_P0B = 26 * WP  # piece B: rows 10..25 (chunks 1-2)
# 256-col PE warm-up matmuls: sized to keep the PE busy (clock ramping)
# through the whole initial DMA window — an idle gap before the first conv
# matmul would drop the clock back down.
NWARM = 12


def _build_program():
    nc = bacc.Bacc("TRN2", target_bir_lowering=False, debug=False,
                   num_devices=NCORES)
    xt = nc.dram_tensor("xt", [BPC, C, NPAD], F16, kind="ExternalInput").ap()
    wm = nc.dram_tensor("wm", [C, BPC * WMC], F16, kind="ExternalInput").ap()
    betad = nc.dram_tensor("beta", [NF, BPC], F32, kind="ExternalInput").ap()
    y = nc.dram_tensor("y", [BPC, NF, NPOS], F16, kind="ExternalOutput").ap()

    with tile.TileContext(nc) as tc:
        with (
            tc.tile_pool(name="const", bufs=1) as cpool,
            tc.tile_pool(name="xt", bufs=BPC) as xpool,
            tc.tile_pool(name="ystage", bufs=2) as ypool,
            tc.tile_pool(name="convps", bufs=6, space="PSUM") as convps,
            tc.tile_pool(name="warmps", bufs=1, space="PSUM") as wps,
        ):
            xt_sb = [xpool.tile([C, NPAD], F16, tag="xt", name=f"xt{b}")
                     for b in range(BPC)]
            wm_sb = cpool.tile([C, BPC * WMC], F16, tag="wm")
            beta_sb = cpool.tile([NF, BPC], F32, tag="beta")
            y_sb = [ypool.tile([NF, NPOS], F16, tag="ystage", name=f"yst{b}")
                    for b in range(BPC)]

            # PE warm-up source memset on the otherwise-idle DVE so neither
            # DMA queue is delayed and the warm-up matmuls start immediately.
            warm_src = cpool.tile([C, 256], F16, tag="warmsrc")
            nc.vector.memset(warm_src[:], 0.0)
            warm_ps = wps.tile([NF, 256], F32, tag="warmps")
            for _ in range(NWARM):
                nc.tensor.matmul(warm_ps[:], warm_src[:, 0:NF], warm_src[:],
                                 start=True, stop=True)

            # Startup DMAs. Queue order decides arrival: sample 0's weights
            # and first input rows land first on their queues.
            nc.sync.dma_start(wm_sb[:, 0:WMC], wm[:, 0:WMC])
            nc.gpsimd.dma_start(xt_sb[0][:, :X_P0A], xt[0][:, :X_P0A])
            nc.sync.dma_start(beta_sb[:], betad)
            nc.gpsimd.dma_start(xt_sb[0][:, X_P0A:X_P0B], xt[0][:, X_P0A:X_P0B])
            nc.gpsimd.dma_start(xt_sb[0][:, X_P0B:], xt[0][:, X_P0B:])
            for b in range(1, BPC):
                nc.sync.dma_start(wm_sb[:, b * WMC:(b + 1) * WMC],
                                  wm[:, b * WMC:(b + 1) * WMC])
            for b in range(1, BPC):
                nc.gpsimd.dma_start(xt_sb[b][:], xt[b][:])

            for b in range(BPC):
                xv = xt_sb[b][:].rearrange("p (h w) -> p h w", w=WP)
                wmb = wm_sb[:, b * WMC:(b + 1) * WMC]
                for t in range(NCHUNK):
                    pc = convps.tile([NF, ROWS_PER_CHUNK * W], F32,
                                     tag="conv")
                    for tap in range(TAPS):
                        dy, dx = tap // 3, tap % 3
                        r0 = ROWS_PER_CHUNK * t + dy
                        rhs = xv[:, r0:r0 + ROWS_PER_CHUNK, dx:dx + W]
                        nc.tensor.matmul(pc[:],
                                         wmb[:, NF * tap:NF * (tap + 1)],
                                         rhs, start=(tap == 0),
                                         stop=(tap == TAPS - 1))
                    nc.scalar.activation(y_sb[b][:, 512 * t:512 * (t + 1)],
                                         pc[:], AF.Relu,
                                         bias=beta_sb[:, b:b + 1])
                    if t == 3:
                        nc.sync.dma_start(y[b][:, :2048], y_sb[b][:, :2048])
                    elif t == 6:
                        nc.scalar.dma_start(y[b][:, 2048:3584],
                                            y_sb[b][:, 2048:3584])
                    elif t == 7:
                        # final piece split across two queues to shrink the
                        # serial drain after the last epilogue
                        nc.scalar.dma_start(y[b][:, 3584:3840],
                                            y_sb[b][:, 3584:3840])
                        nc.sync.dma_start(y[b][:, 3840:],
                                          y_sb[b][:, 3840:])

    nc.compile()
    return nc


_PROGRAM = None


def _get_program():
    global _PROGRAM
    if _PROGRAM is None:
        _PROGRAM = _build_program()
    return _PROGRAM


def _prepare_host_inputs(x, reduction_kernel, attention_kernel, conv_kernels,
                         bias, bn_scale, bn_bias, bn_mean, bn_var):
    f = np.float32
    # Channel-major zero-padded fp16 input: [B, C, 66, 66]
    xt = np.zeros((B, C, HP, WP), dtype=np.float16)
    xt[:, :, 1:H + 1, 1:W + 1] = x.transpose(0, 3, 1, 2)
    xt = xt.reshape(B, C, NPAD)

    # Host routing (fp32, matches the reference numerics).
    pool = x.mean(axis=(1, 2), dtype=f)                      # [B, C]
    pr = np.maximum(pool @ reduction_kernel, 0.0)            # [B, r]
    logits = (pr @ attention_kernel) / f(30.0)               # [B, K]
    e = np.exp(logits - logits.max(axis=1, keepdims=True))
    att = (e / e.sum(axis=1, keepdims=True)).astype(f)       # [B, K]

    inv = (bn_scale / np.sqrt(bn_var + f(1e-5))).astype(f)   # [F]
    # Mixed per-sample conv weights with BN scale folded into F:
    # wm[b] = [C, taps*F] fp16.
    bank = (conv_kernels * inv).astype(f)                    # [K,3,3,C,F]
    bank = bank.transpose(3, 0, 1, 2, 4).reshape(C, K, WMC)  # [C, K, 9F]
    wmix = np.einsum('bk,ckw->cbw', att, bank)               # [C, B, 9F]
    wmix = np.ascontiguousarray(wmix, dtype=np.float16)

    # Per-sample epilogue bias: att@bias folded with BN shift. [F, B]
    beta = ((att @ (bias * inv)) + (bn_bias - bn_mean * inv)).astype(f).T
    beta = np.ascontiguousarray(beta)

    in_maps = []
    for cix in range(NCORES):
        sl = slice(cix * BPC, (cix + 1) * BPC)
        in_maps.append({
            "xt": np.ascontiguousarray(xt[sl]),
            "wm": np.ascontiguousarray(
                wmix[:, sl].reshape(C, BPC * WMC)),
            "beta": np.ascontiguousarray(beta[:, sl]),
        })
    return in_maps


def kernel(x, reduction_kernel, attention_kernel, conv_kernels, bias, bn_scale,
           bn_bias, bn_mean, bn_var, _trace=False):
    nc = _get_program()
    in_maps = _prepare_host_inputs(
        np.asarray(x, dtype=np.float32),
        np.asarray(reduction_kernel, np.float32),
        np.asarray(attention_kernel, np.float32),
        np.asarray(conv_kernels, np.float32), np.asarray(bias, np.float32),
        np.asarray(bn_scale, np.float32), np.asarray(bn_bias, np.float32),
        np.asarray(bn_mean, np.float32), np.asarray(bn_var, np.float32))
    if _trace:
        _ensure_ntff_hook()
    res = run_bass_kernel_spmd(nc, in_maps, core_ids=list(range(NCORES)),
                               trace=_trace)
    yt = np.concatenate([res.results[cix]["y"] for cix in range(NCORES)],
                        axis=0)  # [B, F, 4096] fp16
    out = yt.reshape(B, NF, H, W).transpose(0, 2, 3, 1).astype(np.float32)
    out = np.ascontiguousarray(out)
    if _trace:
        return out, res
    return out


# revision 12
# speedup vs baseline: 1.0216x; 1.0216x over previous
"""Self-contained Trainium2 kernel for nn_DynamicConv2D (moe_routing).

Contract: kernel(**inputs) takes FULL unsharded inputs (numpy), returns the
FULL output [32, 64, 64, 128] float32. Internally shards batch across 8
NeuronCores (4 samples each), runs a Bass/Tile kernel via
run_bass_kernel_spmd, and gathers.

The routing network (global pool -> reduce -> relu -> softmax) and the
expert-kernel mixing are tiny (~19M MACs for the whole batch) and run on the
host in fp32; BN scale/bias and the routed bias are folded in there too.
The device kernel is then a pure per-sample 3x3 conv:

  conv  = 9-tap shifted fp16 matmuls accumulated in PSUM, per 512-pos chunk
  out   = Relu(conv + beta)   (ACT epilogue with per-partition bias, fp16 out)

Layout: x is host-transposed to channel-major [C, H, W], zero-padded to
[C, 66, 66], and cast to fp16 so all 9 conv taps are plain access-pattern
offsets; output is produced channel-major [F, H*W] fp16 and host-transposed
back to NHWC f32. Mixed per-sample weights [C, 9*F] fp16 are DMA'd per
sample (same bytes as the expert bank).
"""

import os
import sys

if "/opt/trn_rl_repo" not in sys.path:
    sys.path.insert(0, "/opt/trn_rl_repo")
if not os.environ.get("JAX_PLATFORMS"):
    os.environ["JAX_PLATFORMS"] = "axon"

import numpy as np

import concourse.bacc as bacc
import concourse.bass_utils as _bass_utils
import concourse.tile as tile
from concourse import mybir
from concourse.bass_utils import run_bass_kernel_spmd

# The walrus NEFF epilogue serially clears every semaphore it may have
# allocated (~250 EVENT_SEMAPHOREs across engines, ~9us on-device). This
# kernel's body syncs exclusively through bass-managed semaphores (150+);
# walrus itself only needs a handful, so cap its allocatable range.
if not getattr(_bass_utils, "_ant_max_sem_patch", False):
    _orig_get_walrus_args = _bass_utils.get_walrus_args

    def _patched_get_walrus_args(*args, **kwargs):
        return ["--max-sem-num=16"] + _orig_get_walrus_args(*args, **kwargs)

    _bass_utils.get_walrus_args = _patched_get_walrus_args
    _bass_utils._ant_max_sem_patch = True


def _ensure_ntff_hook():
    """run_bass_kernel_spmd(trace=True) under axon needs antenv.axon_hooks,
    which this image's antenv package lacks. Register an equivalent module
    (ctypes into libaxon_pjrt.so) so profiled runs work."""
    try:
        from antenv import axon_hooks  # noqa: F401
        return
    except ImportError:
        pass
    import contextlib
    import ctypes
    import types

    so_path = os.environ.get("AXON_PJRT_SO", "/opt/axon/libaxon_pjrt.so")
    mod = types.ModuleType("antenv.axon_hooks")
    state = {"hook": None}

    def _make_hook():
        if not os.path.exists(so_path):
            return None
        lib = ctypes.CDLL(so_path)
        if not hasattr(lib, "axon_start_nrt_profile"):
            return None
        lib.axon_start_nrt_profile.argtypes = [
            ctypes.POINTER(ctypes.c_int64), ctypes.c_size_t]
        lib.axon_start_nrt_profile.restype = ctypes.c_int64
        lib.axon_stop_nrt_profile.argtypes = [ctypes.c_char_p]
        lib.axon_stop_nrt_profile.restype = ctypes.c_int64

        @contextlib.contextmanager
        def _hook(output_dir, device_ids):
            import jax
            jax.devices()
            if device_ids:
                ids = (ctypes.c_int64 * len(device_ids))(*device_ids)
                rc = lib.axon_start_nrt_profile(ids, len(device_ids))
            else:
                rc = lib.axon_start_nrt_profile(None, 0)
            if rc != 0:
                raise RuntimeError(f"axon_start_nrt_profile rc={rc}")
            try:
                yield
            finally:
                n = lib.axon_stop_nrt_profile(str(output_dir).encode())
                if n < 0:
                    raise RuntimeError(f"axon_stop_nrt_profile rc={n}")

        return _hook

    def get_axon_ntff_profile_hook():
        if state["hook"] is None:
            state["hook"] = _make_hook()
        return state["hook"]

    def set_axon_ntff_profile_hook(hook):
        state["hook"] = hook

    mod.get_axon_ntff_profile_hook = get_axon_ntff_profile_hook
    mod.set_axon_ntff_profile_hook = set_axon_ntff_profile_hook
    sys.modules["antenv.axon_hooks"] = mod
    try:
        import antenv
        antenv.axon_hooks = mod
    except ImportError:
        pass


F32 = mybir.dt.float32
F16 = mybir.dt.float16
AF = mybir.ActivationFunctionType

B, H, W, C = 32, 64, 64, 128
NCORES = 8
BPC = B // NCORES  # samples per core
HP, WP = H + 2, W + 2  # zero-padded
NPAD = HP * WP  # 4356
NPOS = H * W  # 4096
K = 4  # experts
NF = 128  # output filters
TAPS = 9
WMC = TAPS * NF  # wm columns per sample (1152)
ROWS_PER_CHUNK = 8  # 8 image rows * 64 cols = 512 positions per PSUM chunk
NCHUNK = H // ROWS_PER_CHUNK
X_P0A = 10 * WP  # sample-0 input piece A: rows 0..9 (covers conv chunk 0)
X_P0B = 26 * WP  # piece B: rows 10..25 (chunks 1-2)
# 256-col PE warm-up matmuls: sized to keep the PE busy (clock ramping)
# through the whole initial DMA window — an idle gap before the first conv
# matmul would drop the clock back down.
NWARM = 10
# Conv chunks per sample: (first output row, rows). The last 8-row chunk is
# split in two so the final epilogue+DMA tail after the last matmul is short.
CHUNKS = [(8 * t, 8) for t in range(7)] + [(56, 4), (60, 4)]


def _build_program():
    nc = bacc.Bacc("TRN2", target_bir_lowering=False, debug=False,
                   num_devices=NCORES)
    xt = nc.dram_tensor("xt", [BPC, C, NPAD], F16, kind="ExternalInput").ap()
    wm = nc.dram_tensor("wm", [C, BPC * WMC], F16, kind="ExternalInput").ap()
    betad = nc.dram_tensor("beta", [NF, BPC], F32, kind="ExternalInput").ap()
    y = nc.dram_tensor("y", [BPC, NF, NPOS], F16, kind="ExternalOutput").ap()

    with tile.TileContext(nc) as tc:
        with (
            tc.tile_pool(name="const", bufs=1) as cpool,
            tc.tile_pool(name="xt", bufs=BPC) as xpool,
            tc.tile_pool(name="ystage", bufs=2) as ypool,
            tc.tile_pool(name="convps", bufs=6, space="PSUM") as convps,
            tc.tile_pool(name="warmps", bufs=1, space="PSUM") as wps,
        ):
            xt_sb = [xpool.tile([C, NPAD], F16, tag="xt", name=f"xt{b}")
                     for b in range(BPC)]
            wm_sb = cpool.tile([C, BPC * WMC], F16, tag="wm")
            beta_sb = cpool.tile([NF, BPC], F32, tag="beta")
            y_sb = [ypool.tile([NF, NPOS], F16, tag="ystage", name=f"yst{b}")
                    for b in range(BPC)]

            # PE warm-up source memset on the otherwise-idle DVE so neither
            # DMA queue is delayed and the warm-up matmuls start immediately.
            warm_src = cpool.tile([C, 256], F16, tag="warmsrc")
            nc.vector.memset(warm_src[:], 0.0)
            warm_ps = wps.tile([NF, 256], F32, tag="warmps")
            for _ in range(NWARM):
                nc.tensor.matmul(warm_ps[:], warm_src[:, 0:NF], warm_src[:],
                                 start=True, stop=True)

            # Startup DMAs. The pieces gating the first conv chunk ride the
            # two HWDGE queues (sync, scalar) whose doorbell-to-data latency
            # is ~0.7us lower than gpsimd's SWDGE queue; the bulk input
            # stream goes on gpsimd.
            nc.sync.dma_start(xt_sb[0][:, :X_P0A], xt[0][:, :X_P0A])
            nc.scalar.dma_start(wm_sb[:, 0:6 * NF], wm[:, 0:6 * NF])
            nc.scalar.dma_start(wm_sb[:, 6 * NF:WMC], wm[:, 6 * NF:WMC])
            nc.sync.dma_start(beta_sb[:], betad)
            nc.gpsimd.dma_start(xt_sb[0][:, X_P0A:X_P0B], xt[0][:, X_P0A:X_P0B])
            nc.gpsimd.dma_start(xt_sb[0][:, X_P0B:], xt[0][:, X_P0B:])
            for b in range(1, BPC):
                nc.scalar.dma_start(wm_sb[:, b * WMC:(b + 1) * WMC],
                                    wm[:, b * WMC:(b + 1) * WMC])
            for b in range(1, BPC):
                nc.gpsimd.dma_start(xt_sb[b][:], xt[b][:])

            for b in range(BPC):
                xv = xt_sb[b][:].rearrange("p (h w) -> p h w", w=WP)
                wmb = wm_sb[:, b * WMC:(b + 1) * WMC]
                for t, (r_base, nrows) in enumerate(CHUNKS):
                    c0, nc0 = r_base * W, nrows * W
                    pc = convps.tile([NF, nc0], F32, tag="conv")
                    for tap in range(TAPS):
                        dy, dx = tap // 3, tap % 3
                        r0 = r_base + dy
                        rhs = xv[:, r0:r0 + nrows, dx:dx + W]
                        nc.tensor.matmul(pc[:],
                                         wmb[:, NF * tap:NF * (tap + 1)],
                                         rhs, start=(tap == 0),
                                         stop=(tap == TAPS - 1))
                    nc.scalar.activation(y_sb[b][:, c0:c0 + nc0],
                                         pc[:], AF.Relu,
                                         bias=beta_sb[:, b:b + 1])
                    if t == 3:
                        nc.sync.dma_start(y[b][:, :2048], y_sb[b][:, :2048])
                    elif t == 6:
                        nc.scalar.dma_start(y[b][:, 2048:3584],
                                            y_sb[b][:, 2048:3584])
                    elif t == 7:
                        nc.scalar.dma_start(y[b][:, 3584:3840],
                                            y_sb[b][:, 3584:3840])
                    elif t == 8:
                        nc.sync.dma_start(y[b][:, 3840:],
                                          y_sb[b][:, 3840:])

    nc.compile()
    return nc


_PROGRAM = None


def _get_program():
    global _PROGRAM
    if _PROGRAM is None:
        _PROGRAM = _build_program()
    return _PROGRAM


def _prepare_host_inputs(x, reduction_kernel, attention_kernel, conv_kernels,
                         bias, bn_scale, bn_bias, bn_mean, bn_var):
    f = np.float32
    # Channel-major zero-padded fp16 input: [B, C, 66, 66]
    xt = np.zeros((B, C, HP, WP), dtype=np.float16)
    xt[:, :, 1:H + 1, 1:W + 1] = x.transpose(0, 3, 1, 2)
    xt = xt.reshape(B, C, NPAD)

    # Host routing (fp32, matches the reference numerics).
    pool = x.mean(axis=(1, 2), dtype=f)                      # [B, C]
    pr = np.maximum(pool @ reduction_kernel, 0.0)            # [B, r]
    logits = (pr @ attention_kernel) / f(30.0)               # [B, K]
    e = np.exp(logits - logits.max(axis=1, keepdims=True))
    att = (e / e.sum(axis=1, keepdims=True)).astype(f)       # [B, K]

    inv = (bn_scale / np.sqrt(bn_var + f(1e-5))).astype(f)   # [F]
    # Mixed per-sample conv weights with BN scale folded into F:
    # wm[b] = [C, taps*F] fp16.
    bank = (conv_kernels * inv).astype(f)                    # [K,3,3,C,F]
    bank = bank.transpose(3, 0, 1, 2, 4).reshape(C, K, WMC)  # [C, K, 9F]
    wmix = np.einsum('bk,ckw->cbw', att, bank)               # [C, B, 9F]
    wmix = np.ascontiguousarray(wmix, dtype=np.float16)

    # Per-sample epilogue bias: att@bias folded with BN shift. [F, B]
    beta = ((att @ (bias * inv)) + (bn_bias - bn_mean * inv)).astype(f).T
    beta = np.ascontiguousarray(beta)

    in_maps = []
    for cix in range(NCORES):
        sl = slice(cix * BPC, (cix + 1) * BPC)
        in_maps.append({
            "xt": np.ascontiguousarray(xt[sl]),
            "wm": np.ascontiguousarray(
                wmix[:, sl].reshape(C, BPC * WMC)),
            "beta": np.ascontiguousarray(beta[:, sl]),
        })
    return in_maps


def kernel(x, reduction_kernel, attention_kernel, conv_kernels, bias, bn_scale,
           bn_bias, bn_mean, bn_var, _trace=False):
    nc = _get_program()
    in_maps = _prepare_host_inputs(
        np.asarray(x, dtype=np.float32),
        np.asarray(reduction_kernel, np.float32),
        np.asarray(attention_kernel, np.float32),
        np.asarray(conv_kernels, np.float32), np.asarray(bias, np.float32),
        np.asarray(bn_scale, np.float32), np.asarray(bn_bias, np.float32),
        np.asarray(bn_mean, np.float32), np.asarray(bn_var, np.float32))
    if _trace:
        _ensure_ntff_hook()
    res = run_bass_kernel_spmd(nc, in_maps, core_ids=list(range(NCORES)),
                               trace=_trace)
    yt = np.concatenate([res.results[cix]["y"] for cix in range(NCORES)],
                        axis=0)  # [B, F, 4096] fp16
    out = yt.reshape(B, NF, H, W).transpose(0, 2, 3, 1).astype(np.float32)
    out = np.ascontiguousarray(out)
    if _trace:
        return out, res
    return out
